# revision 1
# baseline (speedup 1.0000x reference)
"""Trainium2 Bass kernel for nn_CannyEdge (16,3,512,512) -> (16,3,512,512).

Math (verified bit-equivalent to the reference on the fixed input):
  - All 3 output channels are identical; out = f(sum over channels of blurred img).
  - Separable gaussian (reflect pad) + separable sobel (zero pad).
  - NMS decisions taken in msq = gx^2+gy^2 domain (sqrt is monotone).
  - is_max_k = [sum over batch+sides of (msq > shifted msq) == 32] (PE-summed).
  - orientation class from signs of p = gx*gy and D = gy^2 - gx^2.
  - out = mag * is_max_cls.

Sharding: spatial row-strips. Core k owns image rows [64k, 64k+64) of ALL 16
samples (the batch-global min of the reference stays core-local). Rows split
into 2 bands of 32; each (sample, band) is a 40-row strip (32 + 4 halo);
strips pack 3-per-tile on partition slots 0/40/80; 6 tiles per band.
Vertical taps/shifts are banded-matrix matmuls on PE (engines are
partition-lockstep); horizontal shifts are free-dim APs on DVE.
"""

import os

import numpy as np

import concourse.bacc as bacc
import concourse.mybir as mybir
from concourse.mybir import AluOpType as Op
from concourse.tile import TileContext
from concourse.bass_utils import run_bass_kernel_spmd

F32 = mybir.dt.float32
AF = mybir.ActivationFunctionType

B, C, H, W = 16, 3, 512, 512
NCORES = 8
ROWS = H // NCORES          # 64 output rows per core
SH = 32                     # block output rows
HALO = 4
SIN = SH + 2 * HALO         # 40 strip rows
PACK = 3
NBANDS = 2
TPB = 6                     # tiles per band: 5 full + 1 single
NT = NBANDS * TPB
NP = PACK * SIN             # 120 partitions used
BFREE = TPB * W             # 3072
FREE = NT * W               # 6144
NVAR = 4                    # matrix variants: (band h) x (full | single)

KSIZE, SIGMA = 5, 1.4
PAIRS = [(0, 1), (-1, 1), (-1, 0), (-1, -1)]  # E, NE, N, NW
SKIP_PHASEB = bool(os.environ.get("CANNY_SKIP_PHASEB"))
NPAIRS = int(os.environ.get("CANNY_NPAIRS", "4"))
SKIP_STAGE_DEFAULT = int(os.environ.get("CANNY_STAGE", "4"))
USE_POOL = bool(os.environ.get("CANNY_USE_POOL"))

MATNAMES = ("vb", "vs", "vsn", "dv", "dv2", "sel", "shu", "shd")


def _gauss1d():
    half = (KSIZE - 1) * 0.5
    x = np.linspace(-half, half, KSIZE, dtype=np.float32)
    pdf = np.exp(np.float32(-0.5) * (x / np.float32(SIGMA)) ** 2).astype(np.float32)
    return (pdf / pdf.sum()).astype(np.float32)


def _slots(t):
    return [3 * t + j for j in range(PACK) if 3 * t + j < B]


def _band_lhsT(core, h, nslots, taps, offs, mode, out_lo, out_hi):
    """lhsT (K=NP, M=NP) for a vertical conv: out[m] = sum_k lhsT[k,m] x[k]."""
    M = np.zeros((NP, NP), np.float32)
    gr0 = ROWS * core + SH * h - HALO
    for j in range(nslots):
        for io in range(out_lo, out_hi + 1):
            if not (0 <= gr0 + io < H):
                continue  # out row outside image -> column stays 0
            for tap, d in zip(taps, offs):
                g = gr0 + io + d
                if 0 <= g < H:
                    isrc = io + d
                elif mode == "reflect":
                    g2 = -g if g < 0 else 2 * (H - 1) - g
                    isrc = g2 - gr0
                else:
                    continue
                M[SIN * j + isrc, SIN * j + io] += tap
    return M


def _sel3(nslots):
    """V accumulate+replicate: out[40j'+i] += sum_j bits[40j+i], i in 4..35."""
    M = np.zeros((NP, NP), np.float32)
    for j in range(nslots):
        for jp in range(PACK):
            for i in range(HALO, HALO + SH):
                M[SIN * j + i, SIN * jp + i] = 1.0
    return M


def _build_core_inputs(img, core):
    g = _gauss1d()
    k0, k1, k2 = float(g[0]), float(g[1]), float(g[2])

    chin = np.zeros((C, NP, FREE), np.float32)
    for h in range(NBANDS):
        gr0 = ROWS * core + SH * h - HALO
        for t in range(TPB):
            T = TPB * h + t
            for j, s in enumerate(_slots(t)):
                lo = max(0, gr0)
                hi = min(H, gr0 + SIN)
                chin[:, SIN * j + (lo - gr0):SIN * j + (hi - gr0),
                     T * W:(T + 1) * W] = img[s, :, lo:hi, :]

    # matrix variants: v = 2*h + (1 if single-strip tile else 0)
    mats = {n: np.zeros((NVAR, NP, NP), np.float32) for n in MATNAMES}
    for h in range(NBANDS):
        for single in (0, 1):
            v = 2 * h + single
            ns = 1 if single else PACK
            mats["vb"][v] = _band_lhsT(core, h, ns, [k0, k1, k2, k1, k0],
                                       [-2, -1, 0, 1, 2], "reflect", 2, SIN - 3)
            mats["vs"][v] = _band_lhsT(core, h, ns, [1.0, 2.0, 1.0],
                                       [-1, 0, 1], "zero", 3, SIN - 4)
            mats["dv"][v] = _band_lhsT(core, h, ns, [1.0, -1.0],
                                       [-1, 1], "zero", 3, SIN - 4)
            mats["vsn"][v] = -mats["vs"][v]
            mats["dv2"][v] = 2.0 * mats["dv"][v]
            mats["shu"][v] = _band_lhsT(core, h, ns, [1.0], [-1], "zero",
                                        HALO, HALO + SH - 1)
            mats["shd"][v] = _band_lhsT(core, h, ns, [1.0], [1], "zero",
                                        HALO, HALO + SH - 1)
            mats["sel"][v] = _sel3(ns)

    def tr(a):  # (NVAR,NP,NP) -> (NP, NVAR*NP)
        return np.ascontiguousarray(a.transpose(1, 0, 2).reshape(NP, NVAR * NP))

    out = {n: tr(mats[n]) for n in MATNAMES}
    out["chin"] = chin
    return out


def _build_bass(reps=1, stage=None, npairs=None, skipb=None):
    STAGE = SKIP_STAGE_DEFAULT if stage is None else stage
    NPAIRS_ = NPAIRS if npairs is None else npairs
    SKIPB = SKIP_PHASEB if skipb is None else skipb
    g = _gauss1d()
    r0 = float(g[0] / g[1])           # k0/k1
    r1 = float(g[1] / g[2])           # k1/k2
    sc = float(g[2]) * float(g[2])    # k2^2 folded into sqrt

    nc = bacc.Bacc("TRN2", target_bir_lowering=False, debug=False,
                   num_devices=NCORES)

    chin = nc.dram_tensor("chin", [C, NP, FREE], F32, kind="ExternalInput").ap()
    dmats = {n: nc.dram_tensor(n, [NP, NVAR * NP], F32,
                               kind="ExternalInput").ap() for n in MATNAMES}
    outp = nc.dram_tensor("outp", [NP, FREE], F32, kind="ExternalOutput").ap()

    with TileContext(nc) as tc:
        with (
            tc.tile_pool(name="const", bufs=1) as cpool,
            tc.tile_pool(name="chp", bufs=1) as chpool,
            tc.tile_pool(name="persist", bufs=1) as ppool,
            tc.tile_pool(name="work", bufs=2) as wpool,
            tc.tile_pool(name="bits", bufs=1) as bpool,
            tc.tile_pool(name="pb", bufs=1) as pbpool,
            tc.tile_pool(name="ptv", bufs=1, space="PSUM") as ptv,
            tc.tile_pool(name="pgx", bufs=1, space="PSUM") as pgx,
            tc.tile_pool(name="pgy", bufs=1, space="PSUM") as pgy,
            tc.tile_pool(name="pv", bufs=1, space="PSUM") as pvpool,
        ):
            smats = {}
            for name in MATNAMES:
                mt = cpool.tile([NP, NVAR * NP], F32, tag=name, name=f"m_{name}")
                nc.sync.dma_start(out=mt[:], in_=dmats[name])
                smats[name] = mt

            def mat(name, h, t):
                v = 2 * h + (1 if len(_slots(t)) == 1 else 0)
                return smats[name][:, v * NP:(v + 1) * NP]

            msq_s = ppool.tile([NP, BFREE], F32, tag="msq")
            p_s = ppool.tile([NP, BFREE], F32, tag="p")
            d_s = ppool.tile([NP, BFREE], F32, tag="d")
            out_s = ppool.tile([NP, BFREE], F32, tag="out")
            xpl = ppool.tile([NP, 4 * W], F32, tag="xpl")

            def xk(k):
                return xpl[:, k * W:(k + 1) * W]

            for rep in range(reps):
                for h in range(NBANDS):
                    ch_s = chpool.tile([NP, C * BFREE], F32, tag="ch",
                                       name=f"ch{rep}_{h}")
                    for c in range(C):
                        nc.sync.dma_start(
                            out=ch_s[:, c * BFREE:(c + 1) * BFREE],
                            in_=chin[c, :, h * BFREE:(h + 1) * BFREE])

                    vps = [pvpool.tile([NP, W], F32, tag=f"v{k}", name=f"vps{rep}_{h}{k}")
                           for k in range(4)]

                    for t in range(TPB):
                        fs = slice(t * W, (t + 1) * W)

                        def chs(c):
                            return ch_s[:, c * BFREE + t * W:c * BFREE + (t + 1) * W]

                        # vertical gaussian + channel sum (PE)
                        tv = ptv.tile([NP, W], F32, tag="tv", name=f"tv{rep}_{h}{t}")
                        for c in range(C):
                            nc.tensor.matmul(tv[:], mat("vb", h, t), chs(c),
                                             start=(c == 0), stop=(c == C - 1))
                        tvs = wpool.tile([NP, W], F32, tag="tvs", name=f"tvs{rep}_{h}{t}")
                        nc.scalar.activation(tvs[:], tv[:], AF.Copy)

                        # horizontal gaussian (DVE), scaled by 1/k2, reflect pad
                        u1 = wpool.tile([NP, W], F32, bufs=1, tag="u1", name=f"u1{rep}_{h}{t}")
                        nc.vector.tensor_tensor(u1[:, 2:510], tvs[:, 0:508],
                                                tvs[:, 4:512], Op.add)
                        nc.vector.tensor_scalar_mul(u1[:, 0:1], tvs[:, 2:3], 2.0)
                        nc.vector.tensor_tensor(u1[:, 1:2], tvs[:, 1:2],
                                                tvs[:, 3:4], Op.add)
                        nc.vector.tensor_tensor(u1[:, 510:511], tvs[:, 508:509],
                                                tvs[:, 510:511], Op.add)
                        nc.vector.tensor_scalar_mul(u1[:, 511:512],
                                                    tvs[:, 509:510], 2.0)
                        u2 = wpool.tile([NP, W], F32, bufs=1, tag="u2", name=f"u2{rep}_{h}{t}")
                        nc.vector.tensor_tensor(u2[:, 1:511], tvs[:, 0:510],
                                                tvs[:, 2:512], Op.add)
                        nc.vector.tensor_scalar_mul(u2[:, 0:1], tvs[:, 1:2], 2.0)
                        nc.vector.tensor_scalar_mul(u2[:, 511:512],
                                                    tvs[:, 510:511], 2.0)
                        vv = wpool.tile([NP, W], F32, bufs=1, tag="vv", name=f"vv{rep}_{h}{t}")
                        nc.vector.scalar_tensor_tensor(vv[:], u1[:], r0, u2[:],
                                                       Op.mult, Op.add)
                        tt = wpool.tile([NP, W], F32, tag="tt", name=f"tt{rep}_{h}{t}")
                        nc.vector.scalar_tensor_tensor(tt[:], vv[:], r1, tvs[:],
                                                       Op.mult, Op.add)

                        if STAGE < 2:
                            nc.vector.tensor_copy(msq_s[:, fs], tt[:])
                            continue
                        # sobel: horizontal +-1 shifts folded into PE via
                        # column-ranged matmuls (edge cols get zero-pad free)
                        gx = pgx.tile([NP, W], F32, tag="gx", name=f"gx{rep}_{h}{t}")
                        nc.tensor.matmul(gx[:, 1:512], mat("vs", h, t),
                                         tt[:, 0:511], start=True, stop=False)
                        nc.tensor.matmul(gx[:, 0:511], mat("vsn", h, t),
                                         tt[:, 1:512], start=False, stop=True)
                        gy = pgy.tile([NP, W], F32, tag="gy", name=f"gy{rep}_{h}{t}")
                        nc.tensor.matmul(gy[:, 1:512], mat("dv", h, t),
                                         tt[:, 0:511], start=True, stop=False)
                        nc.tensor.matmul(gy[:, 0:511], mat("dv", h, t),
                                         tt[:, 1:512], start=False, stop=False)
                        nc.tensor.matmul(gy[:], mat("dv2", h, t), tt[:],
                                         start=False, stop=True)

                        # squares, msq, p, D
                        sqx = wpool.tile([NP, W], F32, bufs=1, tag="sqx", name=f"sqx{rep}_{h}{t}")
                        nc.scalar.activation(sqx[:], gx[:], AF.Square)
                        sqy = wpool.tile([NP, W], F32, bufs=1, tag="sqy", name=f"sqy{rep}_{h}{t}")
                        nc.scalar.activation(sqy[:], gy[:], AF.Square)
                        gxs = wpool.tile([NP, W], F32, bufs=1, tag="gxs", name=f"gxs{rep}_{h}{t}")
                        nc.scalar.activation(gxs[:], gx[:], AF.Copy)
                        (nc.gpsimd if USE_POOL else nc.vector).tensor_tensor(
                            msq_s[:, fs], sqx[:], sqy[:], Op.add)
                        nc.vector.tensor_tensor(p_s[:, fs], gxs[:], gy[:], Op.mult)
                        (nc.gpsimd if USE_POOL else nc.vector).tensor_tensor(
                            d_s[:, fs], sqy[:], sqx[:], Op.subtract)

                        if STAGE < 3:
                            continue
                        # vertical +-1 shifts of msq (PE; reuses gx/gy PSUM slots)
                        m = msq_s[:, fs]
                        mu = pgx.tile([NP, W], F32, tag="gx", name=f"mu{rep}_{h}{t}")
                        nc.tensor.matmul(mu[:], mat("shu", h, t), m,
                                         start=True, stop=True)   # mu[i]=m[i-1]
                        md = pgy.tile([NP, W], F32, tag="gy", name=f"md{rep}_{h}{t}")
                        nc.tensor.matmul(md[:], mat("shd", h, t), m,
                                         start=True, stop=True)   # md[i]=m[i+1]

                        def vsrc(dr):
                            return m if dr == 0 else (mu if dr == -1 else md)

                        # one-sided pass bits; V accumulation (PE), target sum 32
                        for k, (dr, dc) in enumerate(PAIRS[:NPAIRS_]):
                            for sgn in (1, -1):
                                rdr, rdc = dr * sgn, dc * sgn
                                s = vsrc(rdr)
                                eng = nc.vector
                                bt = bpool.tile([NP, W], F32, tag=f"b{k}{sgn}",
                                                name=f"bt{rep}_{h}{t}{k}{sgn}")
                                lo, hi = max(0, -rdc), W - max(0, rdc)
                                eng.tensor_tensor(
                                    bt[:, lo:hi], s[:, lo + rdc:hi + rdc],
                                    m[:, lo:hi], Op.is_lt)
                                if rdc > 0:
                                    eng.tensor_scalar(
                                        bt[:, W - 1:W], m[:, W - 1:W], 0.0, None,
                                        Op.is_gt)
                                elif rdc < 0:
                                    eng.tensor_scalar(
                                        bt[:, 0:1], m[:, 0:1], 0.0, None, Op.is_gt)
                                nc.tensor.matmul(
                                    vps[k][:], mat("sel", h, t), bt[:],
                                    start=(t == 0 and sgn == 1),
                                    stop=(t == TPB - 1 and sgn == -1))

                    # x planes for this band: all 32 one-sided tests passed
                    for k in (range(NPAIRS_) if STAGE >= 3 else []):
                        nc.vector.tensor_scalar(xk(k), vps[k][:], 32.0, None,
                                                Op.is_equal)

                    # phase B: gate + magnitude (band-level ops)
                    if SKIPB or STAGE < 4:
                        nc.scalar.activation(out_s[:], msq_s[:], AF.Sqrt,
                                             scale=sc)
                        nc.sync.dma_start(
                            out=outp[:, h * BFREE:(h + 1) * BFREE],
                            in_=out_s[:])
                        continue
                    import dataclasses as _dc

                    def rep6(apx):
                        return _dc.replace(apx, ap=[apx.ap[0], [0, TPB],
                                                    apx.ap[1]])

                    def as3(apx):
                        return apx.rearrange("p (s w) -> p s w", w=W)

                    nc.scalar.activation(out_s[:], msq_s[:], AF.Sqrt, scale=sc)
                    vsel = pbpool.tile([NP, BFREE], mybir.dt.uint8, tag="vsel",
                                      name=f"vsel{rep}_{h}")
                    nc.vector.tensor_scalar(vsel[:], p_s[:], 0.0, None,
                                            Op.is_lt)
                    asel = pbpool.tile([NP, BFREE], mybir.dt.uint8, tag="asel",
                                      name=f"asel{rep}_{h}")
                    nc.vector.tensor_scalar(asel[:], d_s[:], 0.0, None,
                                            Op.is_ge)
                    yp = pbpool.tile([NP, BFREE], F32, tag="yp",
                                    name=f"yp{rep}_{h}")
                    nc.vector.tensor_copy(as3(yp[:]), rep6(xk(0)))
                    nc.vector.copy_predicated(as3(yp[:]), as3(asel[:]),
                                              rep6(xk(1)))
                    yn = pbpool.tile([NP, BFREE], F32, tag="yn",
                                    name=f"yn{rep}_{h}")
                    nc.vector.tensor_copy(as3(yn[:]), rep6(xk(3)))
                    nc.vector.copy_predicated(as3(yn[:]), as3(asel[:]),
                                              rep6(xk(2)))
                    nc.vector.copy_predicated(yp[:], vsel[:], yn[:])
                    nc.vector.tensor_tensor(out_s[:], out_s[:], yp[:], Op.mult)

                    nc.sync.dma_start(out=outp[:, h * BFREE:(h + 1) * BFREE],
                                      in_=out_s[:])

    nc.compile()
    return nc


_NC_CACHE = None


def kernel(img):
    global _NC_CACHE
    img = np.ascontiguousarray(np.asarray(img, dtype=np.float32))
    assert img.shape == (B, C, H, W)

    if _NC_CACHE is None:
        _NC_CACHE = _build_bass()
    nc = _NC_CACHE

    in_maps = [_build_core_inputs(img, core) for core in range(NCORES)]
    trace = bool(os.environ.get("CANNY_TRACE"))
    res = run_bass_kernel_spmd(nc, in_maps, core_ids=list(range(NCORES)),
                               trace=trace)
    if trace and res.exec_time_ns is not None:
        print(f"HW exec time: {res.exec_time_ns} ns")
        kernel.last_exec_ns = res.exec_time_ns

    out = np.zeros((B, C, H, W), np.float32)
    for core in range(NCORES):
        o = res.results[core]["outp"]
        for h in range(NBANDS):
            r0b = ROWS * core + SH * h
            for t in range(TPB):
                T = TPB * h + t
                for j, s in enumerate(_slots(t)):
                    blk = o[SIN * j + HALO:SIN * j + HALO + SH,
                            T * W:(T + 1) * W]
                    out[s, :, r0b:r0b + SH, :] = blk[None]
    return out


if __name__ == "__main__":
    img = np.load("/tmp/img.npy")
    out = kernel(img)
    exp = np.load("/tmp/expected.npy")
    d = np.abs(out - exp)
    print("absmax", d.max(), "n>1e-2", (d > 1e-2).sum(),
          "keepmis", ((out != 0) != (exp != 0)).sum())



# revision 9
# speedup vs baseline: 2.2520x; 2.2520x over previous
"""Trainium2 Bass kernel for nn_CannyEdge (16,3,512,512) -> (16,3,512,512).

v2: fp16 on-chip pipeline (validated offline: rel err ~1e-3 vs reference,
ZERO keep-mask flips on the fixed input, robust to +-4ulp perturbation).

Math (all 3 output channels identical; decisions in msq = gx^2+gy^2 domain):
  x = channel-sum(img)                      [host, f32 -> fp16]
  tv = 5-tap vertical gaussian (reflect)    [PE banded matmul, fp16 w/x]
  t  = 5-tap horizontal gaussian (reflect)  [DVE, factored into two 3-tap]
  gx = [1,2,1]^T x [1,0,-1], gy = [1,0,-1]^T x [1,2,1]  (zero pad) [PE]
  sqx,sqy = squares (row-masked via Act scale), msq = sqx+sqy
  pair-max NMS: pb_k = msq > max(msq@+d_k, msq@-d_k); sum over 16 samples
    on PE (sel matmul); keep_k = relu(sum-15)   [Act drain]
  class from signs of p = gx*gy and sqy-sqx; out = sqrt(msq*SC) * keep_class

Sharding: spatial row-strips (batch-global AND stays core-local). Core k owns
image rows [64k,64k+64) of ALL 16 samples; 2 bands x 32 rows; strips of
SIN=40 rows (32+4+4 halo) pack 3 samples per tile on partitions 0/40/80;
6 tiles per band. Vertical taps/shifts are PE banded matmuls (image-boundary
reflect/zero baked into per-core matrices); horizontal shifts are free-dim
APs; msq row-shifts (mu/md) are SBUF->SBUF partition-shifted DMAs.
"""

import os

import numpy as np

import concourse.bacc as bacc
import concourse.mybir as mybir
from concourse.mybir import AluOpType as Op
from concourse.tile import TileContext
from concourse.bass_utils import run_bass_kernel_spmd

F32 = mybir.dt.float32
F16 = mybir.dt.float16
AF = mybir.ActivationFunctionType

B, C, H, W = 16, 3, 512, 512
NCORES = 8
ROWS = H // NCORES          # 64 output rows per core
SH = 32                     # band output rows
HALO = 4
SIN = SH + 2 * HALO         # 40 strip rows
PACK = 3
NBANDS = 2
TPB = 6                     # tiles per band
NT = NBANDS * TPB
NP = PACK * SIN             # 120 partitions used
SEG = W + 4                 # 516: msqz/mu/md segment width (2+512+2)
BFREE = TPB * W             # 3072
NVAR = 4                    # matrix variants: (band h) x (full | single)

KSIZE, SIGMA = 5, 1.4
PAIRS = [(0, 1), (-1, 1), (-1, 0), (-1, -1)]  # E, NE, N, NW

MATNAMES = ("vb", "vs", "vsn", "dv", "dv2", "sel")


def _gauss():
    half = (KSIZE - 1) * 0.5
    x = np.linspace(-half, half, KSIZE, dtype=np.float32)
    pdf = np.exp(np.float32(-0.5) * (x / np.float32(SIGMA)) ** 2).astype(np.float32)
    g = (pdf / pdf.sum()).astype(np.float32)
    q = (g / g[2]).astype(np.float32)          # [q2, q1, 1, q1, q2]
    q2, q1 = float(q[0]), float(q[1])
    s = q1 / q2
    pr = 1.0 / q2 - 2.0
    disc = float(np.sqrt(np.float32(s * s - 4 * pr)))
    a_ = np.float32((s + disc) / 2)            # 3-tap factor taps
    b_ = np.float32((s - disc) / 2)
    a_ = np.float32(np.float16(a_))
    b_ = np.float32(np.float16(b_))
    k0c = np.float32(g[2])
    sc = np.float32((k0c * k0c * np.float32(q2)) ** 2)  # fold into sqrt
    return q, float(a_), float(b_), float(sc)


def _slots(t):
    return [3 * t + j for j in range(PACK) if 3 * t + j < B]


def _band_lhsT(core, h, nslots, taps, offs, mode, out_lo, out_hi):
    """lhsT (K=NP, M=NP) fp16 for a vertical conv: out[m] = sum_k lhsT[k,m] x[k]."""
    M = np.zeros((NP, NP), np.float16)
    gr0 = ROWS * core + SH * h - HALO
    for j in range(nslots):
        for io in range(out_lo, out_hi + 1):
            if not (0 <= gr0 + io < H):
                continue
            for tap, d in zip(taps, offs):
                g = gr0 + io + d
                if 0 <= g < H:
                    isrc = io + d
                elif mode == "reflect":
                    g2 = -g if g < 0 else 2 * (H - 1) - g
                    isrc = g2 - gr0
                else:
                    continue
                M[SIN * j + isrc, SIN * j + io] += np.float16(tap)
    return M


def _sel3(nslots):
    """Sum bits rows 4..35 over slots, replicate to all slots."""
    M = np.zeros((NP, NP), np.float16)
    for j in range(nslots):
        for jp in range(PACK):
            for i in range(HALO, HALO + SH):
                M[SIN * j + i, SIN * jp + i] = 1.0
    return M


def _build_core_inputs(img, core):
    q, _, _, _ = _gauss()
    qh = q.astype(np.float16).astype(np.float32)

    xs = img.sum(axis=1, dtype=np.float32).astype(np.float16)  # (B,H,W)

    chin = np.zeros((NP, NT * W), np.float16)
    for h in range(NBANDS):
        gr0 = ROWS * core + SH * h - HALO
        for t in range(TPB):
            T = TPB * h + t
            for j, s in enumerate(_slots(t)):
                lo = max(0, gr0)
                hi = min(H, gr0 + SIN)
                chin[SIN * j + (lo - gr0):SIN * j + (hi - gr0),
                     T * W:(T + 1) * W] = xs[s, lo:hi, :]

    mats = {n: np.zeros((NVAR, NP, NP), np.float16) for n in MATNAMES}
    for h in range(NBANDS):
        for single in (0, 1):
            v = 2 * h + single
            ns = 1 if single else PACK
            mats["vb"][v] = _band_lhsT(core, h, ns, list(qh),
                                       [-2, -1, 0, 1, 2], "reflect", 2, SIN - 3)
            mats["vs"][v] = _band_lhsT(core, h, ns, [1.0, 2.0, 1.0],
                                       [-1, 0, 1], "zero", 3, SIN - 4)
            mats["vsn"][v] = -mats["vs"][v]
            mats["dv"][v] = _band_lhsT(core, h, ns, [1.0, -1.0],
                                       [-1, 1], "zero", 3, SIN - 4)
            mats["dv2"][v] = 2.0 * mats["dv"][v]
            mats["sel"][v] = _sel3(ns)

    def tr(a):  # (NVAR,NP,NP) -> (NP, NVAR*NP)
        return np.ascontiguousarray(a.transpose(1, 0, 2).reshape(NP, NVAR * NP))

    out = {n: tr(mats[n]) for n in MATNAMES}
    out["chin"] = chin

    # row mask per band: 1 where partition's global row is inside the image
    rmask = np.zeros((NP, NBANDS), np.float32)
    for h in range(NBANDS):
        gr0 = ROWS * core + SH * h - HALO
        for j in range(PACK):
            for i in range(SIN):
                if 0 <= gr0 + i < H:
                    rmask[SIN * j + i, h] = 1.0
    out["rmask"] = rmask
    return out


def _build_bass(reps=1):
    _, a_, b_, sc = _gauss()

    nc = bacc.Bacc("TRN2", target_bir_lowering=False, debug=False,
                   num_devices=NCORES)

    chin = nc.dram_tensor("chin", [NP, NT * W], F16, kind="ExternalInput").ap()
    rmaskD = nc.dram_tensor("rmask", [NP, NBANDS], F32, kind="ExternalInput").ap()
    dmats = {n: nc.dram_tensor(n, [NP, NVAR * NP], F16,
                               kind="ExternalInput").ap() for n in MATNAMES}
    outp = nc.dram_tensor("outp", [NP, NBANDS * BFREE], F16,
                          kind="ExternalOutput").ap()

    with TileContext(nc) as tc:
        with (
            tc.tile_pool(name="const", bufs=1) as cpool,
            tc.tile_pool(name="chp", bufs=1) as chpool,
            tc.tile_pool(name="tile", bufs=2) as wpool,
            tc.tile_pool(name="band", bufs=1) as bpool,
            tc.tile_pool(name="ptv", bufs=1, space="PSUM") as ptv,
            tc.tile_pool(name="pgx", bufs=1, space="PSUM") as pgx,
            tc.tile_pool(name="pgy", bufs=1, space="PSUM") as pgy,
            tc.tile_pool(name="pv", bufs=1, space="PSUM") as pvpool,
        ):
            smats = {}
            for name in MATNAMES:
                mt = cpool.tile([NP, NVAR * NP], F16, tag=name, name=f"m_{name}")
                nc.sync.dma_start(out=mt[:], in_=dmats[name])
                smats[name] = mt
            rmask = cpool.tile([NP, NBANDS], F32, tag="rmask", name="rmask")
            nc.sync.dma_start(out=rmask[:], in_=rmaskD)
            bias15 = cpool.tile([NP, 1], F32, tag="bias15", name="bias15")
            nc.vector.memset(bias15[:], -15.0)
            ch_s = chpool.tile([NP, NT * W], F16, tag="ch", name="ch")
            nc.sync.dma_start(out=ch_s[:], in_=chin)

            def mat(name, h, t):
                v = 2 * h + (1 if len(_slots(t)) == 1 else 0)
                return smats[name][:, v * NP:(v + 1) * NP]

            for rep in range(reps):
                for h in range(NBANDS):
                    msqz = bpool.tile([NP, TPB * SEG], F16, tag="msqz",
                                      name=f"msqz{rep}_{h}")
                    mu = bpool.tile([NP, TPB * SEG], F16, tag="mu",
                                    name=f"mu{rep}_{h}")
                    md = bpool.tile([NP, TPB * SEG], F16, tag="md",
                                    name=f"md{rep}_{h}")
                    p_s = bpool.tile([NP, BFREE], F16, tag="p",
                                     name=f"p{rep}_{h}")
                    asel = bpool.tile([NP, BFREE], mybir.dt.uint8, tag="asel",
                                      name=f"asel{rep}_{h}")

                    for t in range(TPB):
                        xin = ch_s[:, (TPB * h + t) * W:(TPB * h + t + 1) * W]
                        sg = t * SEG

                        tv = ptv.tile([NP, W], F32, tag="tv", name=f"tv{rep}_{h}{t}")
                        nc.tensor.matmul(tv[:], mat("vb", h, t), xin,
                                         start=True, stop=True)
                        tvs = wpool.tile([NP, W + 4], F16, tag="tvs",
                                         name=f"tvs{rep}_{h}{t}")
                        nc.scalar.activation(tvs[:, 2:514], tv[:], AF.Copy)
                        # reflect halo cols (image edges)
                        nc.vector.tensor_copy(tvs[:, 0:1], tvs[:, 4:5])
                        nc.vector.tensor_copy(tvs[:, 1:2], tvs[:, 3:4])
                        nc.vector.tensor_copy(tvs[:, 514:515], tvs[:, 512:513])
                        nc.vector.tensor_copy(tvs[:, 515:516], tvs[:, 511:512])

                        # horizontal gaussian: two 3-tap passes (scale folded)
                        u = wpool.tile([NP, W + 2], F16, tag="u",
                                       name=f"u{rep}_{h}{t}")
                        nc.vector.tensor_tensor(u[:], tvs[:, 0:514],
                                                tvs[:, 2:516], Op.add)
                        v1 = wpool.tile([NP, W + 2], F16, tag="v1",
                                        name=f"v1{rep}_{h}{t}")
                        nc.vector.scalar_tensor_tensor(v1[:], tvs[:, 1:515], a_,
                                                       u[:], Op.mult, Op.add)
                        u2 = wpool.tile([NP, W], F16, tag="u2",
                                        name=f"u2{rep}_{h}{t}")
                        nc.vector.tensor_tensor(u2[:], v1[:, 0:512],
                                                v1[:, 2:514], Op.add)
                        tt = wpool.tile([NP, W], F16, tag="tt",
                                        name=f"tt{rep}_{h}{t}")
                        nc.vector.scalar_tensor_tensor(tt[:], v1[:, 1:513], b_,
                                                       u2[:], Op.mult, Op.add)

                        # sobel on PE (zero pad via matrices + col ranges)
                        gx = pgx.tile([NP, W], F32, tag="gx", name=f"gx{rep}_{h}{t}")
                        nc.tensor.matmul(gx[:, 1:512], mat("vs", h, t),
                                         tt[:, 0:511], start=True, stop=False)
                        nc.tensor.matmul(gx[:, 0:511], mat("vsn", h, t),
                                         tt[:, 1:512], start=False, stop=True)
                        gy = pgy.tile([NP, W], F32, tag="gy", name=f"gy{rep}_{h}{t}")
                        nc.tensor.matmul(gy[:, 1:512], mat("dv", h, t),
                                         tt[:, 0:511], start=True, stop=False)
                        nc.tensor.matmul(gy[:, 0:511], mat("dv", h, t),
                                         tt[:, 1:512], start=False, stop=False)
                        nc.tensor.matmul(gy[:], mat("dv2", h, t), tt[:],
                                         start=False, stop=True)

                        rm = rmask[:, h:h + 1]
                        sqx = wpool.tile([NP, W], F16, tag="sqx",
                                         name=f"sqx{rep}_{h}{t}")
                        nc.scalar.activation(sqx[:], gx[:], AF.Square, scale=rm)
                        sqy = wpool.tile([NP, W], F16, tag="sqy",
                                         name=f"sqy{rep}_{h}{t}")
                        nc.scalar.activation(sqy[:], gy[:], AF.Square, scale=rm)
                        gxs = wpool.tile([NP, W], F16, tag="gxs",
                                         name=f"gxs{rep}_{h}{t}")
                        nc.scalar.activation(gxs[:], gx[:], AF.Copy)
                        gys = wpool.tile([NP, W], F16, tag="gys",
                                         name=f"gys{rep}_{h}{t}")
                        nc.scalar.activation(gys[:], gy[:], AF.Copy)

                        nc.vector.memset(msqz[:, sg:sg + 2], 0.0)
                        nc.vector.memset(msqz[:, sg + 514:sg + 516], 0.0)
                        nc.vector.tensor_tensor(msqz[:, sg + 2:sg + 514],
                                                sqx[:], sqy[:], Op.add)
                        nc.vector.tensor_tensor(p_s[:, t * W:(t + 1) * W],
                                                gxs[:], gys[:], Op.mult)
                        nc.vector.tensor_tensor(asel[:, t * W:(t + 1) * W],
                                                sqy[:], sqx[:], Op.is_ge)

                    # mu/md: partition-shifted copies of msqz (SBUF->SBUF DMA)
                    # mu[p, c+1] = msqz[p-1, c]; md[p, c+1] = msqz[p+1, c]
                    for j in range(PACK):
                        pb_ = SIN * j
                        nc.sync.dma_start(
                            out=mu[pb_ + 1:pb_ + SIN, 1:TPB * SEG],
                            in_=msqz[pb_:pb_ + SIN - 1, 0:TPB * SEG - 1])
                        nc.sync.dma_start(
                            out=md[pb_:pb_ + SIN - 1, 1:TPB * SEG],
                            in_=msqz[pb_ + 1:pb_ + SIN, 0:TPB * SEG - 1])

                    def seg3(plane, lo, wdt):
                        return plane[:].rearrange("p (s c) -> p s c", c=SEG)[
                            :, :, lo:lo + wdt]

                    # pair maxes; mu/md center at idx c+3, msqz center at c+2
                    mxe = bpool.tile([NP, BFREE], F16, tag="mxe", name=f"mxe{rep}{h}")
                    nc.vector.tensor_tensor(
                        mxe[:].rearrange("p (s c) -> p s c", c=W),
                        seg3(msqz, 1, W), seg3(msqz, 3, W), Op.max)
                    mxn = bpool.tile([NP, BFREE], F16, tag="mxn", name=f"mxn{rep}{h}")
                    nc.vector.tensor_tensor(
                        mxn[:].rearrange("p (s c) -> p s c", c=W),
                        seg3(mu, 3, W), seg3(md, 3, W), Op.max)
                    mxne = bpool.tile([NP, BFREE], F16, tag="mxne", name=f"mxne{rep}{h}")
                    nc.vector.tensor_tensor(
                        mxne[:].rearrange("p (s c) -> p s c", c=W),
                        seg3(mu, 4, W), seg3(md, 2, W), Op.max)
                    mxnw = bpool.tile([NP, BFREE], F16, tag="mxnw", name=f"mxnw{rep}{h}")
                    nc.vector.tensor_tensor(
                        mxnw[:].rearrange("p (s c) -> p s c", c=W),
                        seg3(mu, 2, W), seg3(md, 4, W), Op.max)

                    pbts = []
                    for k, mx in enumerate((mxe, mxne, mxn, mxnw)):
                        pbt = bpool.tile([NP, BFREE], F16, tag=f"pb{k}",
                                         name=f"pb{k}_{rep}{h}")
                        nc.vector.tensor_tensor(
                            pbt[:].rearrange("p (s c) -> p s c", c=W),
                            mx[:].rearrange("p (s c) -> p s c", c=W),
                            seg3(msqz, 2, W), Op.is_lt)
                        pbts.append(pbt)

                    # batch-AND on PE: sum over slots+tiles, then relu(s-15)
                    xks = []
                    for k in range(4):
                        vps = pvpool.tile([NP, W], F32, tag=f"v{k}",
                                          name=f"vps{rep}_{h}{k}")
                        for t in range(TPB):
                            nc.tensor.matmul(vps[:], mat("sel", h, t),
                                             pbts[k][:, t * W:(t + 1) * W],
                                             start=(t == 0), stop=(t == TPB - 1))
                        xk = bpool.tile([NP, W], F16, tag=f"xk{k}",
                                        name=f"xk{k}_{rep}{h}")
                        nc.scalar.activation(xk[:], vps[:], AF.Relu, bias=bias15[:, 0:1])
                        xks.append(xk)

                    import dataclasses as _dc

                    def rep6(apx):
                        return _dc.replace(apx, ap=[apx.ap[0], [0, TPB], apx.ap[1]])

                    def as3(apx):
                        return apx.rearrange("p (s w) -> p s w", w=W)

                    vsel = bpool.tile([NP, BFREE], mybir.dt.uint8, tag="vsel",
                                      name=f"vsel{rep}{h}")
                    nc.vector.tensor_scalar(vsel[:], p_s[:], 0.0, None, Op.is_lt)
                    yp = bpool.tile([NP, BFREE], F16, tag="yp", name=f"yp{rep}{h}")
                    nc.vector.tensor_copy(as3(yp[:]), rep6(xks[0][:]))
                    nc.vector.copy_predicated(as3(yp[:]), as3(asel[:]),
                                              rep6(xks[1][:]))
                    # yn = x3 + asel*(x2-x3)  (exact on {0,1}; runs on Pool)
                    d23 = bpool.tile([NP, W], F16, tag="d23", name=f"d23{rep}{h}")
                    nc.gpsimd.tensor_tensor(d23[:], xks[2][:], xks[3][:],
                                            Op.subtract)
                    yn = bpool.tile([NP, BFREE], F16, tag="yn", name=f"yn{rep}{h}")
                    nc.gpsimd.tensor_tensor(as3(yn[:]), as3(asel[:]),
                                            rep6(d23[:]), Op.mult)
                    nc.gpsimd.tensor_tensor(as3(yn[:]), as3(yn[:]),
                                            rep6(xks[3][:]), Op.add)
                    nc.vector.copy_predicated(yp[:], vsel[:], yn[:])

                    mag = bpool.tile([NP, BFREE], F16, tag="mag", name=f"mag{rep}{h}")
                    nc.scalar.activation(mag[:].rearrange("p (s w) -> p s w", w=W),
                                         seg3(msqz, 2, W), AF.Sqrt, scale=sc)
                    out_s = bpool.tile([NP, BFREE], F16, tag="out", name=f"out{rep}{h}")
                    nc.vector.tensor_tensor(out_s[:], mag[:], yp[:], Op.mult)

                    nc.sync.dma_start(out=outp[:, h * BFREE:(h + 1) * BFREE],
                                      in_=out_s[:])

    nc.compile()
    return nc


_NC_CACHE = None
_IN_MAPS_CACHE = {}


def kernel(img):
    global _NC_CACHE
    img = np.ascontiguousarray(np.asarray(img, dtype=np.float32))
    assert img.shape == (B, C, H, W)

    if _NC_CACHE is None:
        _NC_CACHE = _build_bass()
    nc = _NC_CACHE

    in_maps = [_build_core_inputs(img, core) for core in range(NCORES)]
    trace = bool(os.environ.get("CANNY_TRACE"))
    res = run_bass_kernel_spmd(nc, in_maps, core_ids=list(range(NCORES)),
                               trace=trace)
    if trace and res.exec_time_ns is not None:
        print(f"HW exec time: {res.exec_time_ns} ns")
        kernel.last_exec_ns = res.exec_time_ns

    out = np.zeros((B, C, H, W), np.float32)
    for core in range(NCORES):
        o = np.asarray(res.results[core]["outp"], np.float32)
        for h in range(NBANDS):
            r0b = ROWS * core + SH * h
            for t in range(TPB):
                for j, s in enumerate(_slots(t)):
                    blk = o[SIN * j + HALO:SIN * j + HALO + SH,
                            h * BFREE + t * W:h * BFREE + (t + 1) * W]
                    out[s, :, r0b:r0b + SH, :] = blk[None]
    return out


if __name__ == "__main__":
    img = np.load("/tmp/img.npy")
    out = kernel(img)
    exp = np.load("/tmp/expected.npy")
    d = np.abs(out - exp)
    print("absmax", d.max(), "n>1e-2", (d > 1e-2).sum(),
          "keepmis", ((out != 0) != (exp != 0)).sum())


# revision 13
# speedup vs baseline: 2.6790x; 1.1896x over previous
"""Trainium2 Bass kernel for nn_CannyEdge (16,3,512,512) -> (16,3,512,512).

v2: fp16 on-chip pipeline (validated offline: rel err ~1e-3 vs reference,
ZERO keep-mask flips on the fixed input, robust to +-4ulp perturbation).

Math (all 3 output channels identical; decisions in msq = gx^2+gy^2 domain):
  x = channel-sum(img)                      [host, f32 -> fp16]
  tv = 5-tap vertical gaussian (reflect)    [PE banded matmul, fp16 w/x]
  t  = 5-tap horizontal gaussian (reflect)  [DVE, factored into two 3-tap]
  gx = [1,2,1]^T x [1,0,-1], gy = [1,0,-1]^T x [1,2,1]  (zero pad) [PE]
  sqx,sqy = squares (row-masked via Act scale), msq = sqx+sqy
  pair-max NMS: pb_k = msq > max(msq@+d_k, msq@-d_k); sum over 16 samples
    on PE (sel matmul); keep_k = relu(sum-15)   [Act drain]
  class from signs of p = gx*gy and sqy-sqx; out = sqrt(msq*SC) * keep_class

Sharding: spatial row-strips (batch-global AND stays core-local). Core k owns
image rows [64k,64k+64) of ALL 16 samples; 2 bands x 32 rows; strips of
SIN=40 rows (32+4+4 halo) pack 3 samples per tile on partitions 0/40/80;
6 tiles per band. Vertical taps/shifts are PE banded matmuls (image-boundary
reflect/zero baked into per-core matrices); horizontal shifts are free-dim
APs; msq row-shifts (mu/md) are SBUF->SBUF partition-shifted DMAs.
"""

import os

import numpy as np

import concourse.bacc as bacc
import concourse.mybir as mybir
from concourse.mybir import AluOpType as Op
from concourse.tile import TileContext
from concourse.bass_utils import run_bass_kernel_spmd

F32 = mybir.dt.float32
F16 = mybir.dt.float16
AF = mybir.ActivationFunctionType

B, C, H, W = 16, 3, 512, 512
NCORES = 8
ROWS = H // NCORES          # 64 output rows per core
SH = 32                     # band output rows
HALO = 4
SIN = SH + 2 * HALO         # 40 strip rows
PACK = 3
NBANDS = 2
TPB = 6                     # tiles per band
NT = NBANDS * TPB
NP = PACK * SIN             # 120 partitions used
SEG = W + 4                 # 516: msqz/mu/md segment width (2+512+2)
BFREE = TPB * W             # 3072
NVAR = 4                    # matrix variants: (band h) x (full | single)

KSIZE, SIGMA = 5, 1.4
PAIRS = [(0, 1), (-1, 1), (-1, 0), (-1, -1)]  # E, NE, N, NW

MATNAMES = ("vb", "vs", "vsn", "dv", "dv2", "sel")


def _gauss():
    half = (KSIZE - 1) * 0.5
    x = np.linspace(-half, half, KSIZE, dtype=np.float32)
    pdf = np.exp(np.float32(-0.5) * (x / np.float32(SIGMA)) ** 2).astype(np.float32)
    g = (pdf / pdf.sum()).astype(np.float32)
    q = (g / g[2]).astype(np.float32)          # [q2, q1, 1, q1, q2]
    q2, q1 = float(q[0]), float(q[1])
    s = q1 / q2
    pr = 1.0 / q2 - 2.0
    disc = float(np.sqrt(np.float32(s * s - 4 * pr)))
    a_ = np.float32((s + disc) / 2)            # 3-tap factor taps
    b_ = np.float32((s - disc) / 2)
    a_ = np.float32(np.float16(a_))
    b_ = np.float32(np.float16(b_))
    k0c = np.float32(g[2])
    sc = np.float32((k0c * k0c * np.float32(q2)) ** 2)  # fold into sqrt
    return q, float(a_), float(b_), float(sc)


def _slots(t):
    return [3 * t + j for j in range(PACK) if 3 * t + j < B]


def _band_lhsT(core, h, nslots, taps, offs, mode, out_lo, out_hi):
    """lhsT (K=NP, M=NP) fp16 for a vertical conv: out[m] = sum_k lhsT[k,m] x[k]."""
    M = np.zeros((NP, NP), np.float16)
    gr0 = ROWS * core + SH * h - HALO
    for j in range(nslots):
        for io in range(out_lo, out_hi + 1):
            if not (0 <= gr0 + io < H):
                continue
            for tap, d in zip(taps, offs):
                g = gr0 + io + d
                if 0 <= g < H:
                    isrc = io + d
                elif mode == "reflect":
                    g2 = -g if g < 0 else 2 * (H - 1) - g
                    isrc = g2 - gr0
                else:
                    continue
                M[SIN * j + isrc, SIN * j + io] += np.float16(tap)
    return M


def _sel3(nslots):
    """Sum bits rows 4..35 over slots, replicate to all slots."""
    M = np.zeros((NP, NP), np.float16)
    for j in range(nslots):
        for jp in range(PACK):
            for i in range(HALO, HALO + SH):
                M[SIN * j + i, SIN * jp + i] = 1.0
    return M


def _build_core_inputs(img, core):
    q, _, _, _ = _gauss()
    qh = q.astype(np.float16).astype(np.float32)

    xs = img.sum(axis=1, dtype=np.float32).astype(np.float16)  # (B,H,W)

    chin = np.zeros((NP, NT * W), np.float16)
    for h in range(NBANDS):
        gr0 = ROWS * core + SH * h - HALO
        for t in range(TPB):
            T = TPB * h + t
            for j, s in enumerate(_slots(t)):
                lo = max(0, gr0)
                hi = min(H, gr0 + SIN)
                chin[SIN * j + (lo - gr0):SIN * j + (hi - gr0),
                     T * W:(T + 1) * W] = xs[s, lo:hi, :]

    mats = {n: np.zeros((NVAR, NP, NP), np.float16) for n in MATNAMES}
    for h in range(NBANDS):
        for single in (0, 1):
            v = 2 * h + single
            ns = 1 if single else PACK
            mats["vb"][v] = _band_lhsT(core, h, ns, list(qh),
                                       [-2, -1, 0, 1, 2], "reflect", 2, SIN - 3)
            mats["vs"][v] = _band_lhsT(core, h, ns, [1.0, 2.0, 1.0],
                                       [-1, 0, 1], "zero", 3, SIN - 4)
            mats["vsn"][v] = -mats["vs"][v]
            mats["dv"][v] = _band_lhsT(core, h, ns, [1.0, -1.0],
                                       [-1, 1], "zero", 3, SIN - 4)
            mats["dv2"][v] = 2.0 * mats["dv"][v]
            mats["sel"][v] = _sel3(ns)

    def tr(a):  # (NVAR,NP,NP) -> (NP, NVAR*NP)
        return np.ascontiguousarray(a.transpose(1, 0, 2).reshape(NP, NVAR * NP))

    out = {n: tr(mats[n]) for n in MATNAMES}
    out["chin"] = chin

    # row mask per band: 1 where partition's global row is inside the image
    rmask = np.zeros((NP, NBANDS), np.float32)
    for h in range(NBANDS):
        gr0 = ROWS * core + SH * h - HALO
        for j in range(PACK):
            for i in range(SIN):
                if 0 <= gr0 + i < H:
                    rmask[SIN * j + i, h] = 1.0
    out["rmask"] = rmask
    return out


def _build_bass(reps=1):
    _, a_, b_, sc = _gauss()

    nc = bacc.Bacc("TRN2", target_bir_lowering=False, debug=False,
                   num_devices=NCORES)

    chin = nc.dram_tensor("chin", [NP, NT * W], F16, kind="ExternalInput").ap()
    rmaskD = nc.dram_tensor("rmask", [NP, NBANDS], F32, kind="ExternalInput").ap()
    dmats = {n: nc.dram_tensor(n, [NP, NVAR * NP], F16,
                               kind="ExternalInput").ap() for n in MATNAMES}
    outp = nc.dram_tensor("outp", [NP, NBANDS * BFREE], F16,
                          kind="ExternalOutput").ap()

    with TileContext(nc) as tc:
        with (
            tc.tile_pool(name="const", bufs=1) as cpool,
            tc.tile_pool(name="chp", bufs=1) as chpool,
            tc.tile_pool(name="tile", bufs=2) as wpool,
            tc.tile_pool(name="band", bufs=1) as bpool,
            tc.tile_pool(name="ptv", bufs=1, space="PSUM") as ptv,
            tc.tile_pool(name="pgx", bufs=1, space="PSUM") as pgx,
            tc.tile_pool(name="pgy", bufs=2, space="PSUM") as pgy,
            tc.tile_pool(name="pv", bufs=1, space="PSUM") as pvpool,
        ):
            smats = {}
            for name in MATNAMES:
                mt = cpool.tile([NP, NVAR * NP], F16, tag=name, name=f"m_{name}")
                nc.sync.dma_start(out=mt[:], in_=dmats[name])
                smats[name] = mt
            rmask = cpool.tile([NP, NBANDS], F32, tag="rmask", name="rmask")
            nc.sync.dma_start(out=rmask[:], in_=rmaskD)
            bias15 = cpool.tile([NP, 1], F32, tag="bias15", name="bias15")
            nc.vector.memset(bias15[:], -15.0)
            ch_s = chpool.tile([NP, NT * W], F16, tag="ch", name="ch")
            nc.sync.dma_start(out=ch_s[:], in_=chin)

            def mat(name, h, t):
                v = 2 * h + (1 if len(_slots(t)) == 1 else 0)
                return smats[name][:, v * NP:(v + 1) * NP]

            for rep in range(reps):
                for h in range(NBANDS):
                    msqz = bpool.tile([NP, TPB * SEG], F16, tag="msqz",
                                      name=f"msqz{rep}_{h}")
                    mu = bpool.tile([NP, TPB * SEG], F16, tag="mu",
                                    name=f"mu{rep}_{h}")
                    md = bpool.tile([NP, TPB * SEG], F16, tag="md",
                                    name=f"md{rep}_{h}")
                    p_s = bpool.tile([NP, BFREE], F16, tag="p",
                                     name=f"p{rep}_{h}")
                    mz3 = msqz[:].rearrange("p (s c) -> p s c", c=SEG)
                    nc.vector.memset(mz3[:, :, 0:2], 0.0)
                    nc.vector.memset(mz3[:, :, 514:516], 0.0)
                    asel = bpool.tile([NP, BFREE], mybir.dt.uint8, tag="asel",
                                      name=f"asel{rep}_{h}")

                    for t in range(TPB):
                        xin = ch_s[:, (TPB * h + t) * W:(TPB * h + t + 1) * W]
                        sg = t * SEG

                        tv = ptv.tile([NP, W], F32, tag="tv", name=f"tv{rep}_{h}{t}")
                        nc.tensor.matmul(tv[:], mat("vb", h, t), xin,
                                         start=True, stop=True)
                        tvs = wpool.tile([NP, W + 4], F16, tag="tvs",
                                         name=f"tvs{rep}_{h}{t}")
                        nc.scalar.activation(tvs[:, 2:514], tv[:], AF.Copy)
                        # reflect halo cols (image edges): {0<-4, 514<-512} and
                        # {1<-3, 515<-511} as two strided 2-element copies
                        import dataclasses as _dcl

                        def two(apx, step):
                            return _dcl.replace(
                                apx, ap=[apx.ap[0], [step, 2], [1, 1]])

                        nc.vector.tensor_copy(two(tvs[:, 0:1], 514),
                                              two(tvs[:, 4:5], 508))
                        nc.vector.tensor_copy(two(tvs[:, 1:2], 514),
                                              two(tvs[:, 3:4], 508))

                        # horizontal gaussian: two 3-tap passes (scale folded)
                        u = wpool.tile([NP, W + 2], F16, tag="u",
                                       name=f"u{rep}_{h}{t}")
                        nc.vector.tensor_tensor(u[:], tvs[:, 0:514],
                                                tvs[:, 2:516], Op.add)
                        v1 = wpool.tile([NP, W + 2], F16, tag="v1",
                                        name=f"v1{rep}_{h}{t}")
                        nc.vector.scalar_tensor_tensor(v1[:], tvs[:, 1:515], a_,
                                                       u[:], Op.mult, Op.add)
                        u2 = wpool.tile([NP, W], F16, tag="u2",
                                        name=f"u2{rep}_{h}{t}")
                        nc.vector.tensor_tensor(u2[:], v1[:, 0:512],
                                                v1[:, 2:514], Op.add)
                        tt = wpool.tile([NP, W], F16, tag="tt",
                                        name=f"tt{rep}_{h}{t}")
                        nc.vector.scalar_tensor_tensor(tt[:], v1[:, 1:513], b_,
                                                       u2[:], Op.mult, Op.add)

                        # sobel on PE (zero pad via matrices + col ranges)
                        gx = pgx.tile([NP, W], F32, tag="gx", name=f"gx{rep}_{h}{t}")
                        nc.tensor.matmul(gx[:, 1:512], mat("vs", h, t),
                                         tt[:, 0:511], start=True, stop=False)
                        nc.tensor.matmul(gx[:, 0:511], mat("vsn", h, t),
                                         tt[:, 1:512], start=False, stop=True)
                        gy = pgy.tile([NP, W], F32, tag="gy", name=f"gy{rep}_{h}{t}")
                        nc.tensor.matmul(gy[:, 1:512], mat("dv", h, t),
                                         tt[:, 0:511], start=True, stop=False)
                        nc.tensor.matmul(gy[:, 0:511], mat("dv", h, t),
                                         tt[:, 1:512], start=False, stop=False)
                        nc.tensor.matmul(gy[:], mat("dv2", h, t), tt[:],
                                         start=False, stop=True)

                        rm = rmask[:, h:h + 1]
                        sqx = wpool.tile([NP, W], F16, tag="sqx",
                                         name=f"sqx{rep}_{h}{t}")
                        nc.scalar.activation(sqx[:], gx[:], AF.Square, scale=rm)
                        sqy = wpool.tile([NP, W], F16, tag="sqy",
                                         name=f"sqy{rep}_{h}{t}")
                        nc.scalar.activation(sqy[:], gy[:], AF.Square, scale=rm)
                        gxs = wpool.tile([NP, W], F16, tag="gxs",
                                         name=f"gxs{rep}_{h}{t}")
                        nc.scalar.activation(gxs[:], gx[:], AF.Copy)

                        nc.vector.tensor_tensor(msqz[:, sg + 2:sg + 514],
                                                sqx[:], sqy[:], Op.add)
                        nc.vector.tensor_tensor(p_s[:, t * W:(t + 1) * W],
                                                gxs[:], gy[:], Op.mult)
                        nc.vector.tensor_tensor(asel[:, t * W:(t + 1) * W],
                                                sqy[:], sqx[:], Op.is_ge)

                    # mu/md: partition-shifted copies of msqz (SBUF->SBUF DMA)
                    # mu[p, c+1] = msqz[p-1, c]; md[p, c+1] = msqz[p+1, c]
                    for j in range(PACK):
                        pb_ = SIN * j
                        nc.sync.dma_start(
                            out=mu[pb_ + 1:pb_ + SIN, 1:TPB * SEG],
                            in_=msqz[pb_:pb_ + SIN - 1, 0:TPB * SEG - 1])
                        nc.sync.dma_start(
                            out=md[pb_:pb_ + SIN - 1, 1:TPB * SEG],
                            in_=msqz[pb_ + 1:pb_ + SIN, 0:TPB * SEG - 1])

                    def seg3(plane, lo, wdt):
                        return plane[:].rearrange("p (s c) -> p s c", c=SEG)[
                            :, :, lo:lo + wdt]

                    # pair maxes; mu/md center at idx c+3, msqz center at c+2
                    mxe = bpool.tile([NP, BFREE], F16, tag="mxe", name=f"mxe{rep}{h}")
                    nc.vector.tensor_tensor(
                        mxe[:].rearrange("p (s c) -> p s c", c=W),
                        seg3(msqz, 1, W), seg3(msqz, 3, W), Op.max)
                    mxn = bpool.tile([NP, BFREE], F16, tag="mxn", name=f"mxn{rep}{h}")
                    nc.vector.tensor_tensor(
                        mxn[:].rearrange("p (s c) -> p s c", c=W),
                        seg3(mu, 3, W), seg3(md, 3, W), Op.max)
                    mxne = bpool.tile([NP, BFREE], F16, tag="mxne", name=f"mxne{rep}{h}")
                    nc.vector.tensor_tensor(
                        mxne[:].rearrange("p (s c) -> p s c", c=W),
                        seg3(mu, 4, W), seg3(md, 2, W), Op.max)
                    mxnw = bpool.tile([NP, BFREE], F16, tag="mxnw", name=f"mxnw{rep}{h}")
                    nc.vector.tensor_tensor(
                        mxnw[:].rearrange("p (s c) -> p s c", c=W),
                        seg3(mu, 2, W), seg3(md, 4, W), Op.max)

                    pbts = []
                    for k, mx in enumerate((mxe, mxne, mxn, mxnw)):
                        pbt = bpool.tile([NP, BFREE], F16, tag=f"pb{k}",
                                         name=f"pb{k}_{rep}{h}")
                        nc.vector.tensor_tensor(
                            pbt[:].rearrange("p (s c) -> p s c", c=W),
                            mx[:].rearrange("p (s c) -> p s c", c=W),
                            seg3(msqz, 2, W), Op.is_lt)
                        pbts.append(pbt)

                    # batch-AND on PE: sum over slots+tiles, then relu(s-15)
                    xks = []
                    for k in range(4):
                        vps = pvpool.tile([NP, W], F32, tag=f"v{k}",
                                          name=f"vps{rep}_{h}{k}")
                        for t in range(TPB):
                            nc.tensor.matmul(vps[:], mat("sel", h, t),
                                             pbts[k][:, t * W:(t + 1) * W],
                                             start=(t == 0), stop=(t == TPB - 1))
                        xk = bpool.tile([NP, W], F16, tag=f"xk{k}",
                                        name=f"xk{k}_{rep}{h}")
                        nc.scalar.activation(xk[:], vps[:], AF.Relu, bias=bias15[:, 0:1])
                        xks.append(xk)

                    import dataclasses as _dc

                    def rep6(apx):
                        return _dc.replace(apx, ap=[apx.ap[0], [0, TPB], apx.ap[1]])

                    def as3(apx):
                        return apx.rearrange("p (s w) -> p s w", w=W)

                    vsel = bpool.tile([NP, BFREE], mybir.dt.uint8, tag="vsel",
                                      name=f"vsel{rep}{h}")
                    nc.vector.tensor_scalar(vsel[:], p_s[:], 0.0, None, Op.is_lt)
                    d01 = bpool.tile([NP, W], F16, tag="d01", name=f"d01{rep}{h}")
                    nc.gpsimd.tensor_tensor(d01[:], xks[1][:], xks[0][:],
                                            Op.subtract)
                    yp = bpool.tile([NP, BFREE], F16, tag="yp", name=f"yp{rep}{h}")
                    nc.gpsimd.tensor_tensor(as3(yp[:]), as3(asel[:]),
                                            rep6(d01[:]), Op.mult)
                    nc.gpsimd.tensor_tensor(as3(yp[:]), as3(yp[:]),
                                            rep6(xks[0][:]), Op.add)
                    # yn = x3 + asel*(x2-x3)  (exact on {0,1}; runs on Pool)
                    d23 = bpool.tile([NP, W], F16, tag="d23", name=f"d23{rep}{h}")
                    nc.gpsimd.tensor_tensor(d23[:], xks[2][:], xks[3][:],
                                            Op.subtract)
                    yn = bpool.tile([NP, BFREE], F16, tag="yn", name=f"yn{rep}{h}")
                    nc.gpsimd.tensor_tensor(as3(yn[:]), as3(asel[:]),
                                            rep6(d23[:]), Op.mult)
                    nc.gpsimd.tensor_tensor(as3(yn[:]), as3(yn[:]),
                                            rep6(xks[3][:]), Op.add)
                    nc.vector.copy_predicated(yp[:], vsel[:], yn[:])

                    mag = bpool.tile([NP, BFREE], F16, tag="mag", name=f"mag{rep}{h}")
                    nc.scalar.activation(mag[:].rearrange("p (s w) -> p s w", w=W),
                                         seg3(msqz, 2, W), AF.Sqrt, scale=sc)
                    out_s = bpool.tile([NP, BFREE], F16, tag="out", name=f"out{rep}{h}")
                    nc.vector.tensor_tensor(out_s[:], mag[:], yp[:], Op.mult)

                    nc.sync.dma_start(out=outp[:, h * BFREE:(h + 1) * BFREE],
                                      in_=out_s[:])

    nc.compile()
    return nc


_NC_CACHE = None
_IN_MAPS_CACHE = {}


def kernel(img):
    global _NC_CACHE
    img = np.ascontiguousarray(np.asarray(img, dtype=np.float32))
    assert img.shape == (B, C, H, W)

    if _NC_CACHE is None:
        _NC_CACHE = _build_bass()
    nc = _NC_CACHE

    in_maps = [_build_core_inputs(img, core) for core in range(NCORES)]
    trace = bool(os.environ.get("CANNY_TRACE"))
    res = run_bass_kernel_spmd(nc, in_maps, core_ids=list(range(NCORES)),
                               trace=trace)
    if trace and res.exec_time_ns is not None:
        print(f"HW exec time: {res.exec_time_ns} ns")
        kernel.last_exec_ns = res.exec_time_ns

    out = np.zeros((B, C, H, W), np.float32)
    for core in range(NCORES):
        o = np.asarray(res.results[core]["outp"], np.float32)
        for h in range(NBANDS):
            r0b = ROWS * core + SH * h
            for t in range(TPB):
                for j, s in enumerate(_slots(t)):
                    blk = o[SIN * j + HALO:SIN * j + HALO + SH,
                            h * BFREE + t * W:h * BFREE + (t + 1) * W]
                    out[s, :, r0b:r0b + SH, :] = blk[None]
    return out


if __name__ == "__main__":
    img = np.load("/tmp/img.npy")
    out = kernel(img)
    exp = np.load("/tmp/expected.npy")
    d = np.abs(out - exp)
    print("absmax", d.max(), "n>1e-2", (d > 1e-2).sum(),
          "keepmis", ((out != 0) != (exp != 0)).sum())
